# revision 39
# baseline (speedup 1.0000x reference)
"""Trainium2 Bass kernel for DeformBottleneckBlock (DCNv2 bottleneck).

Sharding: 8 cores = (batch b in 0..3) x (H-half in 0..1); each core computes
output rows [lo, lo+50) of one image. Fully data-parallel, no collectives.

Position ordering inside stages C/D uses pi-order: output position
p = 128*n + 16*t + q  (t in 0..8 chunk, n in 0..40, q in 0..16), column
j = 16*n + q within chunk t.  This makes the dma_gather's 16-partition
wrapped index layout reachable with contiguous DMAs (the f32 baseline's
element-granularity relayout storm was >half the runtime).  The host
pre-permutes the residual input and un-permutes the output.

Per-core pipeline:
  A) conv1 1x1 (bf16, bn1 folded, bias via indicator channel) -> out1
     channel-major bf16 cmv [128, 2, 58, 108]; PE transposes build the
     shingled token-major buffer tm[x_pad, y, 512ch] (1KB/token).
  B) offset conv 3x3 (im2col shifted views, PSUM-accumulated) ->
     om [27,5120] bf16 (linear p), plus om_pi (pi-ordered copy).
  B2) om -> DRAM -> xbar DMA-transpose -> om128 [128, (27,40)] (128-wrap);
     idx pipeline on [128,360] tiles -> wrapped+replicated gather indices
     via one contiguous DRAM bounce.  Maps pipeline on [72,640] tiles
     (row = (k,t)) -> bilinear corner weight maps w00..w11 (validity- and
     sigmoid-mask-folded), packed in maps4.
  C) per (t,k): one merged dma_gather (top+bot rows, 1280 idxs, 1KB
     tokens), PE broadcast of the 4 weight maps via selM, 4 muls + 3 adds
     -> s0 bf16, PSUM-accumulated matmuls (w2, bn2 folded) -> relu -> out2.
  D) conv3 1x1 (bf16) + residual add (via identity matmul of bf16 x) +
     bias (via ones-row matmul) + relu -> out (bf16, pi-ordered).
"""

import numpy as np
import ml_dtypes

B, CIN, H, W = 4, 1024, 100, 100
CB, COUT, KOFF = 256, 1024, 27

PAD = 4
RSTRIP = 58
WPAD = 108
NPOS = 5120          # 5000 valid + 120 fake
NT = 8               # chunks (t)
NJ = 640             # positions per chunk
NN = 40              # n per chunk
NVALID = 5000
NKT = 72             # (k, t) rows for maps


def _build_program():
    import concourse.bacc as bacc
    import concourse.mybir as mybir
    from concourse.tile import TileContext
    from concourse.bass import ts
    from concourse.masks import make_identity

    dt = mybir.dt
    AF = mybir.ActivationFunctionType
    ALU = mybir.AluOpType
    f32, bf16, i16, i32 = dt.float32, dt.bfloat16, dt.int16, dt.int32

    nc = bacc.Bacc("TRN2", target_bir_lowering=False, num_swdge_queues=4)

    xs_d = nc.dram_tensor("xs", [128, 8 * RSTRIP * W], bf16, kind="ExternalInput")
    ind_d = nc.dram_tensor("ind", [1, RSTRIP * W], bf16, kind="ExternalInput")
    w1T_d = nc.dram_tensor("w1T", [128, 8 * CB], bf16, kind="ExternalInput")
    w1b_d = nc.dram_tensor("w1b", [1, CB], bf16, kind="ExternalInput")
    woff_d = nc.dram_tensor("woff", [128, 9 * 2 * KOFF], bf16, kind="ExternalInput")
    boff_d = nc.dram_tensor("boff", [KOFF, 1], f32, kind="ExternalInput")
    w2_d = nc.dram_tensor("w2", [128, 9 * 2 * CB], bf16, kind="ExternalInput")
    b2_d = nc.dram_tensor("b2", [128, 2], f32, kind="ExternalInput")
    w3_d = nc.dram_tensor("w3", [128, 2 * COUT], bf16, kind="ExternalInput")
    b3_d = nc.dram_tensor("b3", [128, 8], f32, kind="ExternalInput")
    by128_d = nc.dram_tensor("by128", [128, 9 * NN], f32, kind="ExternalInput")
    bx128_d = nc.dram_tensor("bx128", [128, 9 * NN], f32, kind="ExternalInput")
    byM_d = nc.dram_tensor("byM", [NKT, NJ], f32, kind="ExternalInput")
    bxM_d = nc.dram_tensor("bxM", [NKT, NJ], f32, kind="ExternalInput")
    vb_d = nc.dram_tensor("vb", [NKT, 4], f32, kind="ExternalInput")
    sel_d = nc.dram_tensor("sel", [NKT, NKT * 128], bf16, kind="ExternalInput")
    xres_d = nc.dram_tensor("xres", [8, 128, NPOS], bf16, kind="ExternalInput")
    dom_d = nc.dram_tensor("dom", [1, 1152 * 128], bf16)
    didx_d = nc.dram_tensor("didx", [1, 128 * 720], i16)
    out_d = nc.dram_tensor("out", [8, 128, NPOS], bf16, kind="ExternalOutput")

    with TileContext(nc) as tc:
        with tc.tile_pool(name="persist", bufs=1) as pp, \
             tc.tile_pool(name="io", bufs=2) as iop:

            tm = pp.tile([128, RSTRIP, 4, 128], bf16)
            req = pp.tile([128, NT * 9 * 2 * NN], i16)  # gather idxs, wrapped+rep
            maps4 = pp.tile([NKT, 4, NJ], bf16)
            selM = pp.tile([NKT, NKT * 128], bf16)
            w2s = pp.tile([128, 9 * 2 * CB], bf16)
            w3s = pp.tile([128, 2 * COUT], bf16)
            b2 = pp.tile([128, 2], f32)
            b3 = pp.tile([128, 8], f32)
            ident = pp.tile([128, 128], bf16)
            nc.sync.dma_start(out=selM, in_=sel_d[:, :])
            nc.sync.dma_start(out=w2s, in_=w2_d[:, :])
            nc.sync.dma_start(out=w3s, in_=w3_d[:, :])
            nc.sync.dma_start(out=b2, in_=b2_d[:, :])
            nc.sync.dma_start(out=b3, in_=b3_d[:, :])
            make_identity(nc, ident)

            # only the partitions the gather can touch but the transposes
            # never write need zeroing (x0_pad in 108..110, x-wrap 125..127,
            # and the sh=1 shingle's last column); 16-aligned for gpsimd
            nc.gpsimd.memset(tm[96:128, :, :, :], 0)

            with tc.tile_pool(name="omscope", bufs=1) as omp:
                om = omp.tile([KOFF, NPOS], bf16)
                om_pi = omp.tile([KOFF, NPOS], bf16)
                nc.vector.memset(om[:, NVALID:], 0)

                # ======== Stage A: conv1 + tm build ========
                with tc.tile_pool(name="stageab", bufs=1) as ap, \
                     tc.tile_pool(name="xck", bufs=2) as xp, \
                     tc.tile_pool(name="psA", bufs=2, space="PSUM") as psA:

                    cm = ap.tile([128, 2, RSTRIP * WPAD], bf16)
                    cmv0 = cm.rearrange("p c (r w) -> p c r w", w=WPAD)
                    nc.vector.memset(cmv0[:, :, :, 0:PAD], 0)
                    nc.vector.memset(cmv0[:, :, :, PAD + W:], 0)
                    w1T = ap.tile([128, 8, CB], bf16)
                    nc.sync.dma_start(out=w1T, in_=w1T_d[:, :].rearrange(
                        "p (k c) -> p k c", k=8))
                    w1b = ap.tile([1, CB], bf16)
                    nc.sync.dma_start(out=w1b, in_=w1b_d[:, :])
                    woffT = ap.tile([128, 9, 2, KOFF], bf16)
                    nc.sync.dma_start(out=woffT, in_=woff_d[:, :].rearrange(
                        "p (t c k) -> p t c k", t=9, c=2))
                    boff = ap.tile([KOFF, 1], f32)
                    nc.sync.dma_start(out=boff, in_=boff_d[:, :])

                    cmv = cm.rearrange("p c (r w) -> p c r w", w=WPAD)

                    chunks = [(4 * i, 4) for i in range(14)] + [(56, 2)]
                    for (r0, nrows) in chunks:
                        npos = nrows * W
                        xt = xp.tile([128, 8, 4 * W], bf16, tag="xchunk")
                        nc.sync.dma_start(
                            out=xt[:, :, :npos],
                            in_=xs_d[:, :].rearrange(
                                "p (k n) -> p k n", k=8)[:, :, r0 * W:r0 * W + npos])
                        indt = xp.tile([1, 4 * W], bf16, tag="indchunk")
                        nc.sync.dma_start(out=indt[:, :npos],
                                          in_=ind_d[:, r0 * W:r0 * W + npos])
                        for mt in range(2):
                            ps = psA.tile([128, 4 * W], f32, tag="convps")
                            for kt in range(8):
                                nc.tensor.matmul(ps[:, :npos], w1T[:, kt, ts(mt, 128)],
                                                 xt[:, kt, :npos],
                                                 start=(kt == 0), stop=False)
                            nc.tensor.matmul(ps[:, :npos], w1b[:, ts(mt, 128)],
                                             indt[:, :npos], start=False, stop=True)
                            nc.scalar.activation(
                                cmv[:, mt, r0:r0 + nrows, PAD:PAD + W],
                                ps[:, :npos].rearrange("p (r w) -> p r w", w=W),
                                AF.Relu)

                    # ======== Stage B: offset conv (before transposes so the
                    # B2 pipeline overlaps the tm build) ========
                    for rc in range(10):
                        r0 = rc * 5
                        npos = 5 * W
                        ps = psA.tile([KOFF, 5 * W], f32, tag="omps")
                        first = True
                        for tap in range(9):
                            ti, tj = divmod(tap, 3)
                            rhs = cmv[:, :, r0 + 3 + ti:r0 + 3 + ti + 5,
                                      PAD + tj - 1:PAD + tj - 1 + W]
                            for ct in range(2):
                                nc.tensor.matmul(
                                    ps.rearrange("p (r w) -> p r w", w=W),
                                    woffT[:, tap, ct, :], rhs[:, ct],
                                    start=first, stop=(tap == 8 and ct == 1))
                                first = False
                        nc.scalar.activation(om[:, rc * npos:(rc + 1) * npos], ps,
                                             AF.Identity, bias=boff[:, :])

                    for y in range(RSTRIP):
                        for ct in range(2):
                            for sh in range(2):
                                ncols = WPAD if sh == 0 else WPAD - 1
                                pst = psA.tile([128, 128], bf16, tag="tpose")
                                nc.tensor.transpose(pst[:ncols, :],
                                                    cmv[:, ct, y, sh:sh + ncols],
                                                    ident)
                                if (y + ct) % 2 == 0:
                                    nc.scalar.copy(tm[:ncols, y, 2 * sh + ct, :],
                                                   pst[:ncols, :])
                                else:
                                    nc.vector.tensor_copy(
                                        tm[:ncols, y, 2 * sh + ct, :],
                                        pst[:ncols, :])

                # pi-ordered copy of om: col t*640 + n*16 + q  <- p = 128n+16t+q
                nc.vector.tensor_copy(
                    om_pi.rearrange("p (t n q) -> p n t q", t=NT, n=NN, q=16),
                    om.rearrange("p (n t q) -> p n t q", n=NN, t=NT, q=16))

                # ======== Stage B2a: om -> DRAM -> xbar 128-wrap ========
                ztail = pp.tile([72, 128], bf16)
                nc.vector.memset(ztail[:, :], 0)
                nc.sync.dma_start(
                    out=dom_d[0, KOFF * NPOS:].rearrange("(r p) -> r p", p=128),
                    in_=ztail[:, :])
                nc.sync.dma_start(
                    out=dom_d[0, :KOFF * NPOS].rearrange("(r p) -> r p", r=KOFF),
                    in_=om[:, :])
                om128 = pp.tile([128, 1152], bf16)
                nc.sync.dma_start_transpose(
                    out=om128,
                    in_=dom_d[0, :].rearrange("(r c) -> r c", c=128))

                # ======== Stage B2c: maps pipeline on [72, 640] ========
                with tc.tile_pool(name="fieldsc", bufs=1) as fc:
                    _tn = [0]

                    def T(tag, d=f32):
                        _tn[0] += 1
                        return fc.tile([NKT, NJ], d, tag=tag,
                                       name=f"fld_{tag}_{_tn[0]}")

                    dyM = T("pA", bf16)
                    dxM = T("pB", bf16)
                    mrM = T("pC", bf16)
                    opi = om_pi.rearrange("p (t c) -> p t c", t=NT)
                    for k in range(9):
                        nc.sync.dma_start(
                            out=dyM[8 * k:8 * k + 8, :], in_=opi[2 * k:2 * k + 1])
                        nc.sync.dma_start(
                            out=dxM[8 * k:8 * k + 8, :],
                            in_=opi[2 * k + 1:2 * k + 2])
                        nc.sync.dma_start(
                            out=mrM[8 * k:8 * k + 8, :], in_=opi[18 + k:19 + k])
                    byM = T("pD")
                    nc.sync.dma_start(out=byM, in_=byM_d[:, :])
                    bxM = T("pE")
                    nc.sync.dma_start(out=bxM, in_=bxM_d[:, :])
                    vb = fc.tile([NKT, 4], f32)
                    nc.sync.dma_start(out=vb, in_=vb_d[:, :])

                    dyMf = T("pF")
                    nc.vector.tensor_copy(dyMf, dyM)               # pA free
                    ayy = T("pA")
                    nc.vector.tensor_add(ayy, dyMf, byM)           # pF, pD free
                    ayi = T("pF", i32)
                    nc.vector.tensor_copy(ayi, ayy)
                    ayf = T("pD")
                    nc.vector.tensor_copy(ayf, ayi)                # pF free
                    wyh = T("pF")
                    nc.vector.tensor_sub(wyh, ayy, ayf)            # pA free
                    msig = T("pA")
                    nc.scalar.activation(msig, mrM, AF.Sigmoid)    # pC free
                    dxMf = T("pC")
                    nc.vector.tensor_copy(dxMf, dxM)               # pB free
                    bxx = T("pB")
                    nc.vector.tensor_add(bxx, dxMf, bxM)           # pC, pE free
                    bxi = T("pC", i32)
                    nc.vector.tensor_copy(bxi, bxx)
                    bxf = T("pE")
                    nc.vector.tensor_copy(bxf, bxi)                # pC free
                    wxh = T("pC")
                    nc.vector.tensor_sub(wxh, bxx, bxf)            # pB free

                    def cmp_range(dst, src, lo_ap, hi_ap, tmp):
                        nc.vector.tensor_scalar(tmp, src, lo_ap, None, ALU.is_ge)
                        nc.vector.tensor_scalar(dst, src, hi_ap, None, ALU.is_le)
                        nc.vector.tensor_mul(dst, dst, tmp)

                    tmp = T("pB")
                    vy0 = T("pG")
                    cmp_range(vy0, ayf, vb[:, 0:1], vb[:, 1:2], tmp)
                    vy1 = T("pH")
                    cmp_range(vy1, ayf, vb[:, 2:3], vb[:, 3:4], tmp)  # pD free
                    atop = T("pD")
                    nc.vector.tensor_scalar(atop, wyh, -1.0, 0.5, ALU.mult, ALU.add)
                    nc.vector.tensor_mul(atop, atop, msig)
                    nc.vector.tensor_mul(atop, atop, vy0)             # pG free
                    abot = T("pG")
                    nc.vector.tensor_scalar(abot, wyh, 0.5, None, ALU.add)
                    nc.vector.tensor_mul(abot, abot, msig)
                    nc.vector.tensor_mul(abot, abot, vy1)   # pF, pA, pH free
                    vx0 = T("pA")
                    cmp_range(vx0, bxf, 12.0, 111.0, tmp)
                    vx1 = T("pF")
                    cmp_range(vx1, bxf, 11.0, 110.0, tmp)             # pE free
                    c0 = T("pE")
                    nc.vector.tensor_scalar(c0, wxh, -1.0, 0.5, ALU.mult, ALU.add)
                    nc.vector.tensor_mul(c0, c0, vx0)                 # pA free
                    c1 = T("pA")
                    nc.vector.tensor_scalar(c1, wxh, 0.5, None, ALU.add)
                    nc.vector.tensor_mul(c1, c1, vx1)
                    nc.vector.tensor_mul(maps4[:, 0, :], atop, c0)
                    nc.vector.tensor_mul(maps4[:, 1, :], atop, c1)
                    nc.vector.tensor_mul(maps4[:, 2, :], abot, c0)
                    nc.vector.tensor_mul(maps4[:, 3, :], abot, c1)

                # ======== Stage B2b: idx pipeline on [128, 360] ========
                with tc.tile_pool(name="idxp", bufs=1) as ip:
                    om128v = om128[:, 0:720].rearrange(
                        "u (r a n) -> u r a n", r=9, a=2)
                    by128 = ip.tile([128, 9, NN], f32)
                    nc.sync.dma_start(out=by128, in_=by128_d[:, :].rearrange(
                        "p (k n) -> p k n", k=9))
                    bx128 = ip.tile([128, 9, NN], f32)
                    nc.sync.dma_start(out=bx128, in_=bx128_d[:, :].rearrange(
                        "p (k n) -> p k n", k=9))

                    dyf = ip.tile([128, 9, NN], f32, tag="iA")
                    nc.vector.tensor_copy(dyf, om128v[:, :, 0, :])
                    ayy = ip.tile([128, 9, NN], f32, tag="iB")
                    nc.vector.tensor_add(ayy, dyf, by128)
                    ayi = ip.tile([128, 9, NN], i32, tag="iA")
                    nc.vector.tensor_copy(ayi, ayy)
                    ayf = ip.tile([128, 9, NN], f32, tag="iB")
                    nc.vector.tensor_copy(ayf, ayi)
                    dxf = ip.tile([128, 9, NN], f32, tag="iC")
                    nc.vector.tensor_copy(dxf, om128v[:, :, 1, :])
                    bxx = ip.tile([128, 9, NN], f32, tag="iD")
                    nc.vector.tensor_add(bxx, dxf, bx128)
                    bxi = ip.tile([128, 9, NN], i32, tag="iC")
                    nc.vector.tensor_copy(bxi, bxx)
                    bxf = ip.tile([128, 9, NN], f32, tag="iD")
                    nc.vector.tensor_copy(bxf, bxi)
                    idxf = ip.tile([128, 9, NN], f32, tag="iA")
                    nc.vector.tensor_scalar(idxf, ayf, 128.0, -1032.0,
                                            ALU.mult, ALU.add)
                    nc.vector.tensor_add(idxf, idxf, bxf)
                    idx_t = ip.tile([128, 9, NN], i16, tag="iE")
                    nc.vector.tensor_copy(idx_t, idxf)
                    nc.vector.tensor_scalar(idxf, idxf, 128.0, None, ALU.add)
                    idx_b = ip.tile([128, 9, NN], i16, tag="iF")
                    nc.vector.tensor_copy(idx_b, idxf)

                    # bounce: didx[u*720 + k*80 + tb*40 + n]
                    dv = didx_d[0, :].rearrange("(u k b n) -> u k b n",
                                                u=128, k=9, b=2, n=NN)
                    nc.sync.dma_start(out=dv[:, :, 0, :], in_=idx_t)
                    nc.sync.dma_start(out=dv[:, :, 1, :], in_=idx_b)
                    # req[16g+q, t, k, tb, n] = didx[(16t+q)*720 + k*80+tb*40+n]
                    rin = didx_d[0, :].rearrange("(t q c) -> q t c", t=NT, q=16)
                    for g in range(8):
                        nc.sync.dma_start(
                            out=req[16 * g:16 * g + 16, :].rearrange(
                                "q (t c) -> q t c", t=NT),
                            in_=rin)

            # ======== Stage C + D (interleaved per t) ========
            out2 = pp.tile([128, 2, NPOS], bf16)
            with tc.tile_pool(name="stagec", bufs=1) as cp, \
                 tc.tile_pool(name="gath", bufs=2) as gp, \
                 tc.tile_pool(name="gath3", bufs=2) as gp3, \
                 tc.tile_pool(name="sK", bufs=2) as skp, \
                 tc.tile_pool(name="sK1", bufs=1) as skp1, \
                 tc.tile_pool(name="psC", bufs=1, space="PSUM") as psC, \
                 tc.tile_pool(name="psW", bufs=1, space="PSUM") as psW, \
                 tc.tile_pool(name="psD", bufs=1, space="PSUM") as psD:

                tmflat = tm.rearrange("p r a b -> p (r a b)")
                segs = [(0, 512), (512, 128)]
                for t in range(NT):
                    s0s = []
                    for k in range(9):
                        r = k * NT + t
                        gt = gp3.tile([128, 4, NJ], bf16, tag="gt")
                        gb = gp3.tile([128, 4, NJ], bf16, tag="gb")
                        for tb, gdst in ((0, gt), (1, gb)):
                            nc.gpsimd.dma_gather(
                                out_ap=gdst[:, :, :],
                                in_ap=tmflat,
                                idxs_ap=req[:, t * 720 + k * 80 + tb * 40:
                                            t * 720 + k * 80 + (tb + 1) * 40],
                                num_idxs=NJ, num_idxs_reg=NJ,
                                elem_size=512, transpose=True,
                                sbuf_tokens_per_rank=128,
                                sbuf_free_dim_per_rank=1024,
                                queue_num=(2 * (t * 9 + k) + tb) % 4)
                        wsb = gp.tile([128, 4, NJ], bf16, tag="wsb",
                                      name=f"wsb_{t}_{k}")
                        for half in range(2):
                            pw = psW.tile([128, 2 * NJ], f32, tag="wps",
                                          name=f"wps_{t}_{k}_{half}")
                            rhs = maps4[:, 2 * half:2 * half + 2, :].rearrange(
                                "p a c -> p (a c)")
                            for (s0c, sn) in ((0, 512), (512, 512), (1024, 256)):
                                nc.tensor.matmul(
                                    pw[:, s0c:s0c + sn],
                                    selM[:, r * 128:(r + 1) * 128],
                                    rhs[:, s0c:s0c + sn],
                                    start=True, stop=True)
                            nc.scalar.copy(
                                wsb[:, 2 * half:2 * half + 2, :].rearrange(
                                    "p a c -> p (a c)"), pw)

                        s0 = (skp if k < 2 else skp1).tile(
                            [128, 2, NJ], bf16, tag=f"s0_{k}",
                            name=f"s0_{t}_{k}")
                        scr = gp.tile([128, 4, NJ], bf16, tag="scr")

                        def dup(pair):
                            return pair.unsqueeze(2).broadcast_to(
                                [128, 2, 2, NJ])

                        scrv = scr.rearrange("p (x c) j -> p x c j", x=2)
                        nc.vector.tensor_mul(
                            scrv, gt.rearrange("p (x c) j -> p x c j", x=2),
                            dup(wsb[:, 0:2, :]))
                        nc.vector.tensor_add(s0, scrv[:, 0], scrv[:, 1])
                        nc.vector.tensor_mul(
                            scrv, gb.rearrange("p (x c) j -> p x c j", x=2),
                            dup(wsb[:, 2:4, :]))
                        nc.vector.tensor_add(s0, s0, scrv[:, 0])
                        nc.vector.tensor_add(s0, s0, scrv[:, 1])
                        s0s.append(s0)
                    for mt in range(2):
                        acc = psC.tile([128, NJ], f32, tag="dacc",
                                       name=f"dacc_{t}_{mt}")
                        for (c0s, cn) in segs:
                            for k in range(9):
                                for ct in range(2):
                                    lhsT = w2s[:, (k * 2 + ct) * CB + mt * 128:
                                               (k * 2 + ct) * CB + (mt + 1) * 128]
                                    nc.tensor.matmul(
                                        acc[:, c0s:c0s + cn], lhsT,
                                        s0s[k][:, ct, c0s:c0s + cn],
                                        start=(k == 0 and ct == 0),
                                        stop=(k == 8 and ct == 1))
                        nc.scalar.activation(out2[:, mt, t * NJ:(t + 1) * NJ],
                                             acc, AF.Relu,
                                             bias=b2[:, mt:mt + 1])
                    # ---- Stage D for chunk t ----
                    n0 = t * NJ
                    for mt in range(8):
                        xr = iop.tile([128, NJ], bf16, tag="xres")
                        nc.sync.dma_start(out=xr,
                                          in_=xres_d[mt, :, n0:n0 + NJ])
                        ps = psD.tile([128, NJ], f32, tag="c3ps",
                                      name=f"c3ps_{t}_{mt}")
                        for (c0s, cn) in segs:
                            for ct in range(2):
                                nc.tensor.matmul(
                                    ps[:, c0s:c0s + cn],
                                    w3s[:, ct * COUT + mt * 128:
                                        ct * COUT + (mt + 1) * 128],
                                    out2[:, ct, n0 + c0s:n0 + c0s + cn],
                                    start=(ct == 0), stop=False)
                            nc.tensor.matmul(ps[:, c0s:c0s + cn], ident,
                                             xr[:, c0s:c0s + cn],
                                             start=False, stop=True)
                        o = iop.tile([128, NJ], bf16, tag="obuf")
                        nc.scalar.activation(o, ps, AF.Relu,
                                             bias=b3[:, mt:mt + 1])
                        nc.sync.dma_start(out=out_d[mt, :, n0:n0 + NJ], in_=o)

    nc.finalize()
    return nc


_NC_CACHE = None


def _get_nc():
    global _NC_CACHE
    if _NC_CACHE is None:
        _NC_CACHE = _build_program()
    return _NC_CACHE


def _perm():
    """pi-order: column t*640 + 16n + q  ->  p = 128n + 16t + q."""
    t = np.arange(NPOS) // NJ
    j = np.arange(NPOS) % NJ
    n, q = j // 16, j % 16
    return (128 * n + 16 * t + q).astype(np.int64)


def _prep_inputs(x, w1, s1, b1, w_off, b_off, w2, s2, b2, w3, s3, b3):
    bf16 = ml_dtypes.bfloat16
    f32 = np.float32
    x = np.asarray(x, f32)
    w1f = np.asarray(w1, f32) * np.asarray(s1, f32)[:, None]
    # w1T[ci, kt, co]: contraction cin = kt*128 + ci
    w1T = np.ascontiguousarray(
        w1f.T.reshape(8, 128, CB).transpose(1, 0, 2).reshape(128, 8 * CB)
    ).astype(bf16)
    w1b = np.asarray(b1, f32)[None, :].astype(bf16)
    w_off = np.asarray(w_off, f32)
    woff = np.zeros((128, 9, 2, KOFF), f32)
    for tap in range(9):
        ti, tj = divmod(tap, 3)
        wt = w_off[:, :, ti, tj]          # [27, 256]
        for ct in range(2):
            woff[:, tap, ct, :] = wt[:, ct * 128:(ct + 1) * 128].T
    woff = woff.reshape(128, 9 * 2 * KOFF).astype(bf16)
    boff = np.asarray(b_off, f32)[:, None]
    w2f = np.asarray(w2, f32) * np.asarray(s2, f32)[:, None, None, None]
    w2pk = np.zeros((128, 9, 2, CB), f32)
    for k in range(9):
        ki, kj = divmod(k, 3)
        wk = w2f[:, :, ki, kj]            # [256 out, 256 in]
        for ct in range(2):
            w2pk[:, k, ct, :] = wk[:, ct * 128:(ct + 1) * 128].T
    w2pk = w2pk.reshape(128, 9 * 2 * CB).astype(bf16)
    b2t = np.ascontiguousarray(np.asarray(b2, f32).reshape(2, 128).T)
    w3f = np.asarray(w3, f32) * np.asarray(s3, f32)[:, None]
    w3pk = np.zeros((128, 2, COUT), f32)
    for ct in range(2):
        w3pk[:, ct, :] = w3f[:, ct * 128:(ct + 1) * 128].T
    w3pk = w3pk.reshape(128, 2 * COUT).astype(bf16)
    b3t = np.ascontiguousarray(np.asarray(b3, f32).reshape(8, 128).T)

    perm = _perm()                        # col -> p
    p_lin = np.arange(NPOS)
    y_loc = np.where(p_lin < NVALID, PAD + p_lin // W, 20).astype(f32)
    x_pad = np.where(p_lin < NVALID, PAD + p_lin % W, 50).astype(f32)

    # [128, 9*40] layout: partition u, col k*40+n -> p = 128n + u
    y_w = y_loc.reshape(NN, 128).T          # [u, n]
    x_w = x_pad.reshape(NN, 128).T
    by128 = np.zeros((128, 9, NN), f32)
    bx128 = np.zeros((128, 9, NN), f32)
    for k in range(9):
        ki, kj = divmod(k, 3)
        by128[:, k, :] = y_w + np.float32(ki - 1 + 7.5)
        bx128[:, k, :] = x_w + np.float32(kj - 1 + 7.5)
    by128 = by128.reshape(128, 9 * NN)
    bx128 = bx128.reshape(128, 9 * NN)

    # [72, 640] layout: row k*8+t, col j -> p = perm[t*640+j]
    byM = np.zeros((NKT, NJ), f32)
    bxM = np.zeros((NKT, NJ), f32)
    for k in range(9):
        ki, kj = divmod(k, 3)
        for t in range(NT):
            pp_ = perm[t * NJ:(t + 1) * NJ]
            byM[k * NT + t] = y_loc[pp_] + (ki - 1) + 7.5
            bxM[k * NT + t] = x_pad[pp_] + (kj - 1) + 7.5

    selm = np.zeros((NKT, NKT * 128), bf16)
    for r in range(NKT):
        selm[r, r * 128:(r + 1) * 128] = 1.0

    shared = dict(w1T=w1T, w1b=w1b, woff=woff, boff=boff, w2=w2pk, b2=b2t,
                  w3=w3pk, b3=b3t, by128=by128, bx128=bx128,
                  byM=byM, bxM=bxM, sel=np.asarray(selm))

    in_maps = []
    for core in range(8):
        b, half = core // 2, core % 2
        lo = half * 50
        xs = np.zeros((CIN, RSTRIP, W), f32)
        vlo = max(0, lo - PAD)
        vhi = min(H - 1, lo + 49 + PAD)
        loc0 = vlo - (lo - PAD)
        nrows = vhi - vlo + 1
        xs[:, loc0:loc0 + nrows, :] = x[b, :, vlo:vhi + 1, :]
        # xs_d [128 ci, 8 kt, 5800]
        xs_t = np.ascontiguousarray(
            xs.reshape(8, 128, RSTRIP * W).transpose(1, 0, 2)
        ).reshape(128, 8 * RSTRIP * W).astype(bf16)
        indv = np.zeros((RSTRIP, W), f32)
        indv[loc0:loc0 + nrows, :] = 1.0
        vbm = np.zeros((NKT, 4), f32)
        vbm[:, 0] = loc0 + 8
        vbm[:, 1] = loc0 + nrows - 1 + 8
        vbm[:, 2] = loc0 + 8 - 1
        vbm[:, 3] = loc0 + nrows - 1 + 8 - 1
        # residual, pi-ordered bf16: xres[mt, ci, col] = x[b, mt*128+ci, p(col)]
        xflat = x[b].reshape(CIN, H * W)
        pv = perm.copy()
        src = lo * W + pv                     # absolute flat position
        valid = pv < NVALID
        xr = np.zeros((CIN, NPOS), f32)
        xr[:, valid] = xflat[:, src[valid]]
        xres = np.ascontiguousarray(
            xr.reshape(8, 128, NPOS)).astype(bf16)
        in_maps.append(dict(shared, xs=xs_t,
                            ind=indv.reshape(1, -1).astype(bf16),
                            vb=vbm, xres=xres))
    return in_maps


def kernel(**inputs):
    from concourse.bass_utils import run_bass_kernel_spmd
    nc = _get_nc()
    in_maps = _prep_inputs(**inputs)
    res = run_bass_kernel_spmd(nc, in_maps, core_ids=list(range(8)))
    perm = _perm()
    inv = np.empty(NPOS, np.int64)
    inv[perm] = np.arange(NPOS)
    out = np.zeros((B, COUT, H, W), np.float32)
    for core in range(8):
        b, half = core // 2, core % 2
        lo = half * 50
        o = res.results[core]["out"].astype(np.float32)  # [8, 128, NPOS]
        o = o.reshape(COUT, NPOS)[:, inv[:NVALID]].reshape(COUT, 50, W)
        out[b, :, lo:lo + 50, :] = o
    return out


# revision 40
# speedup vs baseline: 1.0036x; 1.0036x over previous
"""Trainium2 Bass kernel for DeformBottleneckBlock (DCNv2 bottleneck).

Sharding: 8 cores = (batch b in 0..3) x (H-half in 0..1); each core computes
output rows [lo, lo+50) of one image. Fully data-parallel, no collectives.

Position ordering inside stages C/D uses pi-order: output position
p = 128*n + 16*t + q  (t in 0..8 chunk, n in 0..40, q in 0..16), column
j = 16*n + q within chunk t.  This makes the dma_gather's 16-partition
wrapped index layout reachable with contiguous DMAs (the f32 baseline's
element-granularity relayout storm was >half the runtime).  The host
pre-permutes the residual input and un-permutes the output.

Per-core pipeline:
  A) conv1 1x1 (bf16, bn1 folded, bias via indicator channel) -> out1
     channel-major bf16 cmv [128, 2, 58, 108]; PE transposes build the
     shingled token-major buffer tm[x_pad, y, 512ch] (1KB/token).
  B) offset conv 3x3 (im2col shifted views, PSUM-accumulated) ->
     om [27,5120] bf16 (linear p), plus om_pi (pi-ordered copy).
  B2) om -> DRAM -> xbar DMA-transpose -> om128 [128, (27,40)] (128-wrap);
     idx pipeline on [128,360] tiles -> wrapped+replicated gather indices
     via one contiguous DRAM bounce.  Maps pipeline on [72,640] tiles
     (row = (k,t)) -> bilinear corner weight maps w00..w11 (validity- and
     sigmoid-mask-folded), packed in maps4.
  C) per (t,k): one merged dma_gather (top+bot rows, 1280 idxs, 1KB
     tokens), PE broadcast of the 4 weight maps via selM, 4 muls + 3 adds
     -> s0 bf16, PSUM-accumulated matmuls (w2, bn2 folded) -> relu -> out2.
  D) conv3 1x1 (bf16) + residual add (via identity matmul of bf16 x) +
     bias (via ones-row matmul) + relu -> out (bf16, pi-ordered).
"""

import numpy as np
import ml_dtypes

B, CIN, H, W = 4, 1024, 100, 100
CB, COUT, KOFF = 256, 1024, 27

PAD = 4
RSTRIP = 58
WPAD = 108
NPOS = 5120          # 5000 valid + 120 fake
NT = 8               # chunks (t)
NJ = 640             # positions per chunk
NN = 40              # n per chunk
NVALID = 5000
NKT = 72             # (k, t) rows for maps


def _build_program():
    import concourse.bacc as bacc
    import concourse.mybir as mybir
    from concourse.tile import TileContext
    from concourse.bass import ts
    from concourse.masks import make_identity

    dt = mybir.dt
    AF = mybir.ActivationFunctionType
    ALU = mybir.AluOpType
    f32, bf16, i16, i32 = dt.float32, dt.bfloat16, dt.int16, dt.int32

    nc = bacc.Bacc("TRN2", target_bir_lowering=False, num_swdge_queues=4)

    xs_d = nc.dram_tensor("xs", [128, 8 * RSTRIP * W], bf16, kind="ExternalInput")
    ind_d = nc.dram_tensor("ind", [1, RSTRIP * W], bf16, kind="ExternalInput")
    w1T_d = nc.dram_tensor("w1T", [128, 8 * CB], bf16, kind="ExternalInput")
    w1b_d = nc.dram_tensor("w1b", [1, CB], bf16, kind="ExternalInput")
    woff_d = nc.dram_tensor("woff", [128, 9 * 2 * KOFF], bf16, kind="ExternalInput")
    boff_d = nc.dram_tensor("boff", [KOFF, 1], f32, kind="ExternalInput")
    w2_d = nc.dram_tensor("w2", [128, 9 * 2 * CB], bf16, kind="ExternalInput")
    b2_d = nc.dram_tensor("b2", [128, 2], f32, kind="ExternalInput")
    w3_d = nc.dram_tensor("w3", [128, 2 * COUT], bf16, kind="ExternalInput")
    b3_d = nc.dram_tensor("b3", [128, 8], f32, kind="ExternalInput")
    by128_d = nc.dram_tensor("by128", [128, 9 * NN], f32, kind="ExternalInput")
    bx128_d = nc.dram_tensor("bx128", [128, 9 * NN], f32, kind="ExternalInput")
    byM_d = nc.dram_tensor("byM", [NKT, NJ], f32, kind="ExternalInput")
    bxM_d = nc.dram_tensor("bxM", [NKT, NJ], f32, kind="ExternalInput")
    vb_d = nc.dram_tensor("vb", [NKT, 4], f32, kind="ExternalInput")
    sel_d = nc.dram_tensor("sel", [NKT, NKT * 128], bf16, kind="ExternalInput")
    xres_d = nc.dram_tensor("xres", [8, 128, NPOS], bf16, kind="ExternalInput")
    dom_d = nc.dram_tensor("dom", [1, 1152 * 128], bf16)
    didx_d = nc.dram_tensor("didx", [1, 128 * 720], i16)
    out_d = nc.dram_tensor("out", [8, 128, NPOS], bf16, kind="ExternalOutput")

    with TileContext(nc) as tc:
        with tc.tile_pool(name="persist", bufs=1) as pp, \
             tc.tile_pool(name="io", bufs=2) as iop:

            tm = pp.tile([128, RSTRIP, 4, 128], bf16)
            req = pp.tile([128, NT * 9 * 2 * NN], i16)  # gather idxs, wrapped+rep
            maps4 = pp.tile([NKT, 4, NJ], bf16)
            selM = pp.tile([NKT, NKT * 128], bf16)
            w2s = pp.tile([128, 9 * 2 * CB], bf16)
            w3s = pp.tile([128, 2 * COUT], bf16)
            b2 = pp.tile([128, 2], f32)
            b3 = pp.tile([128, 8], f32)
            ident = pp.tile([128, 128], bf16)
            nc.sync.dma_start(out=selM, in_=sel_d[:, :])
            nc.sync.dma_start(out=w2s, in_=w2_d[:, :])
            nc.sync.dma_start(out=w3s, in_=w3_d[:, :])
            nc.sync.dma_start(out=b2, in_=b2_d[:, :])
            nc.sync.dma_start(out=b3, in_=b3_d[:, :])
            make_identity(nc, ident)

            # only the partitions the gather can touch but the transposes
            # never write need zeroing (x0_pad in 108..110, x-wrap 125..127,
            # and the sh=1 shingle's last column); 16-aligned for gpsimd
            nc.gpsimd.memset(tm[96:128, :, :, :], 0)

            with tc.tile_pool(name="omscope", bufs=1) as omp:
                om = omp.tile([KOFF, NPOS], bf16)
                om_pi = omp.tile([KOFF, NPOS], bf16)
                nc.vector.memset(om[:, NVALID:], 0)

                # ======== Stage A: conv1 + tm build ========
                with tc.tile_pool(name="stageab", bufs=1) as ap, \
                     tc.tile_pool(name="xck", bufs=2) as xp, \
                     tc.tile_pool(name="psA", bufs=2, space="PSUM") as psA:

                    cm = ap.tile([128, 2, RSTRIP * WPAD], bf16)
                    cmv0 = cm.rearrange("p c (r w) -> p c r w", w=WPAD)
                    nc.vector.memset(cmv0[:, :, :, 0:PAD], 0)
                    nc.vector.memset(cmv0[:, :, :, PAD + W:], 0)
                    w1T = ap.tile([128, 8, CB], bf16)
                    nc.sync.dma_start(out=w1T, in_=w1T_d[:, :].rearrange(
                        "p (k c) -> p k c", k=8))
                    w1b = ap.tile([1, CB], bf16)
                    nc.sync.dma_start(out=w1b, in_=w1b_d[:, :])
                    woffT = ap.tile([128, 9, 2, KOFF], bf16)
                    nc.sync.dma_start(out=woffT, in_=woff_d[:, :].rearrange(
                        "p (t c k) -> p t c k", t=9, c=2))
                    boff = ap.tile([KOFF, 1], f32)
                    nc.sync.dma_start(out=boff, in_=boff_d[:, :])

                    cmv = cm.rearrange("p c (r w) -> p c r w", w=WPAD)

                    chunks = [(4 * i, 4) for i in range(14)] + [(56, 2)]
                    for (r0, nrows) in chunks:
                        npos = nrows * W
                        xt = xp.tile([128, 8, 4 * W], bf16, tag="xchunk")
                        nc.sync.dma_start(
                            out=xt[:, :, :npos],
                            in_=xs_d[:, :].rearrange(
                                "p (k n) -> p k n", k=8)[:, :, r0 * W:r0 * W + npos])
                        indt = xp.tile([1, 4 * W], bf16, tag="indchunk")
                        nc.sync.dma_start(out=indt[:, :npos],
                                          in_=ind_d[:, r0 * W:r0 * W + npos])
                        for mt in range(2):
                            ps = psA.tile([128, 4 * W], f32, tag="convps")
                            for kt in range(8):
                                nc.tensor.matmul(ps[:, :npos], w1T[:, kt, ts(mt, 128)],
                                                 xt[:, kt, :npos],
                                                 start=(kt == 0), stop=False)
                            nc.tensor.matmul(ps[:, :npos], w1b[:, ts(mt, 128)],
                                             indt[:, :npos], start=False, stop=True)
                            nc.scalar.activation(
                                cmv[:, mt, r0:r0 + nrows, PAD:PAD + W],
                                ps[:, :npos].rearrange("p (r w) -> p r w", w=W),
                                AF.Relu)

                    # ======== Stage B: offset conv (before transposes so the
                    # B2 pipeline overlaps the tm build) ========
                    for rc in range(10):
                        r0 = rc * 5
                        npos = 5 * W
                        ps = psA.tile([KOFF, 5 * W], f32, tag="omps")
                        first = True
                        for tap in range(9):
                            ti, tj = divmod(tap, 3)
                            rhs = cmv[:, :, r0 + 3 + ti:r0 + 3 + ti + 5,
                                      PAD + tj - 1:PAD + tj - 1 + W]
                            for ct in range(2):
                                nc.tensor.matmul(
                                    ps.rearrange("p (r w) -> p r w", w=W),
                                    woffT[:, tap, ct, :], rhs[:, ct],
                                    start=first, stop=(tap == 8 and ct == 1))
                                first = False
                        nc.scalar.activation(om[:, rc * npos:(rc + 1) * npos], ps,
                                             AF.Identity, bias=boff[:, :])

                    for y in range(RSTRIP):
                        for ct in range(2):
                            for sh in range(2):
                                ncols = WPAD if sh == 0 else WPAD - 1
                                pst = psA.tile([128, 128], bf16, tag="tpose")
                                nc.tensor.transpose(pst[:ncols, :],
                                                    cmv[:, ct, y, sh:sh + ncols],
                                                    ident)
                                if (y + ct) % 2 == 0:
                                    nc.scalar.copy(tm[:ncols, y, 2 * sh + ct, :],
                                                   pst[:ncols, :])
                                else:
                                    nc.vector.tensor_copy(
                                        tm[:ncols, y, 2 * sh + ct, :],
                                        pst[:ncols, :])

                # pi-ordered copy of om: col t*640 + n*16 + q  <- p = 128n+16t+q
                nc.vector.tensor_copy(
                    om_pi.rearrange("p (t n q) -> p n t q", t=NT, n=NN, q=16),
                    om.rearrange("p (n t q) -> p n t q", n=NN, t=NT, q=16))

                # ======== Stage B2a: om -> DRAM -> xbar 128-wrap ========
                ztail = pp.tile([72, 128], bf16)
                nc.vector.memset(ztail[:, :], 0)
                nc.sync.dma_start(
                    out=dom_d[0, KOFF * NPOS:].rearrange("(r p) -> r p", p=128),
                    in_=ztail[:, :])
                nc.sync.dma_start(
                    out=dom_d[0, :KOFF * NPOS].rearrange("(r p) -> r p", r=KOFF),
                    in_=om[:, :])
                om128 = pp.tile([128, 1152], bf16)
                nc.sync.dma_start_transpose(
                    out=om128,
                    in_=dom_d[0, :].rearrange("(r c) -> r c", c=128))

                # ======== Stage B2c: maps pipeline on [72, 640] ========
                with tc.tile_pool(name="fieldsc", bufs=1) as fc:
                    _tn = [0]

                    def T(tag, d=f32):
                        _tn[0] += 1
                        return fc.tile([NKT, NJ], d, tag=tag,
                                       name=f"fld_{tag}_{_tn[0]}")

                    dyM = T("pA", bf16)
                    dxM = T("pB", bf16)
                    mrM = T("pC", bf16)
                    opi = om_pi.rearrange("p (t c) -> p t c", t=NT)
                    for k in range(9):
                        nc.sync.dma_start(
                            out=dyM[8 * k:8 * k + 8, :], in_=opi[2 * k:2 * k + 1])
                        nc.sync.dma_start(
                            out=dxM[8 * k:8 * k + 8, :],
                            in_=opi[2 * k + 1:2 * k + 2])
                        nc.sync.dma_start(
                            out=mrM[8 * k:8 * k + 8, :], in_=opi[18 + k:19 + k])
                    byM = T("pD")
                    nc.sync.dma_start(out=byM, in_=byM_d[:, :])
                    bxM = T("pE")
                    nc.sync.dma_start(out=bxM, in_=bxM_d[:, :])
                    vb = fc.tile([NKT, 4], f32)
                    nc.sync.dma_start(out=vb, in_=vb_d[:, :])

                    dyMf = T("pF")
                    nc.vector.tensor_copy(dyMf, dyM)               # pA free
                    ayy = T("pA")
                    nc.vector.tensor_add(ayy, dyMf, byM)           # pF, pD free
                    ayi = T("pF", i32)
                    nc.vector.tensor_copy(ayi, ayy)
                    ayf = T("pD")
                    nc.vector.tensor_copy(ayf, ayi)                # pF free
                    wyh = T("pF")
                    nc.vector.tensor_sub(wyh, ayy, ayf)            # pA free
                    msig = T("pA")
                    nc.scalar.activation(msig, mrM, AF.Sigmoid)    # pC free
                    dxMf = T("pC")
                    nc.vector.tensor_copy(dxMf, dxM)               # pB free
                    bxx = T("pB")
                    nc.vector.tensor_add(bxx, dxMf, bxM)           # pC, pE free
                    bxi = T("pC", i32)
                    nc.vector.tensor_copy(bxi, bxx)
                    bxf = T("pE")
                    nc.vector.tensor_copy(bxf, bxi)                # pC free
                    wxh = T("pC")
                    nc.vector.tensor_sub(wxh, bxx, bxf)            # pB free

                    def cmp_range(dst, src, lo_ap, hi_ap, tmp):
                        nc.vector.tensor_scalar(tmp, src, lo_ap, None, ALU.is_ge)
                        nc.vector.tensor_scalar(dst, src, hi_ap, None, ALU.is_le)
                        nc.vector.tensor_mul(dst, dst, tmp)

                    tmp = T("pB")
                    vy0 = T("pG")
                    cmp_range(vy0, ayf, vb[:, 0:1], vb[:, 1:2], tmp)
                    vy1 = T("pH")
                    cmp_range(vy1, ayf, vb[:, 2:3], vb[:, 3:4], tmp)  # pD free
                    atop = T("pD")
                    nc.vector.tensor_scalar(atop, wyh, -1.0, 0.5, ALU.mult, ALU.add)
                    nc.vector.tensor_mul(atop, atop, msig)
                    nc.vector.tensor_mul(atop, atop, vy0)             # pG free
                    abot = T("pG")
                    nc.vector.tensor_scalar(abot, wyh, 0.5, None, ALU.add)
                    nc.vector.tensor_mul(abot, abot, msig)
                    nc.vector.tensor_mul(abot, abot, vy1)   # pF, pA, pH free
                    vx0 = T("pA")
                    cmp_range(vx0, bxf, 12.0, 111.0, tmp)
                    vx1 = T("pF")
                    cmp_range(vx1, bxf, 11.0, 110.0, tmp)             # pE free
                    c0 = T("pE")
                    nc.vector.tensor_scalar(c0, wxh, -1.0, 0.5, ALU.mult, ALU.add)
                    nc.vector.tensor_mul(c0, c0, vx0)                 # pA free
                    c1 = T("pA")
                    nc.vector.tensor_scalar(c1, wxh, 0.5, None, ALU.add)
                    nc.vector.tensor_mul(c1, c1, vx1)
                    nc.vector.tensor_mul(maps4[:, 0, :], atop, c0)
                    nc.vector.tensor_mul(maps4[:, 1, :], atop, c1)
                    nc.vector.tensor_mul(maps4[:, 2, :], abot, c0)
                    nc.vector.tensor_mul(maps4[:, 3, :], abot, c1)

                # ======== Stage B2b: idx pipeline on [128, 360] ========
                with tc.tile_pool(name="idxp", bufs=1) as ip:
                    om128v = om128[:, 0:720].rearrange(
                        "u (r a n) -> u r a n", r=9, a=2)
                    by128 = ip.tile([128, 9, NN], f32)
                    nc.sync.dma_start(out=by128, in_=by128_d[:, :].rearrange(
                        "p (k n) -> p k n", k=9))
                    bx128 = ip.tile([128, 9, NN], f32)
                    nc.sync.dma_start(out=bx128, in_=bx128_d[:, :].rearrange(
                        "p (k n) -> p k n", k=9))

                    dyf = ip.tile([128, 9, NN], f32, tag="iA")
                    nc.vector.tensor_copy(dyf, om128v[:, :, 0, :])
                    ayy = ip.tile([128, 9, NN], f32, tag="iB")
                    nc.vector.tensor_add(ayy, dyf, by128)
                    ayi = ip.tile([128, 9, NN], i32, tag="iA")
                    nc.vector.tensor_copy(ayi, ayy)
                    ayf = ip.tile([128, 9, NN], f32, tag="iB")
                    nc.vector.tensor_copy(ayf, ayi)
                    dxf = ip.tile([128, 9, NN], f32, tag="iC")
                    nc.vector.tensor_copy(dxf, om128v[:, :, 1, :])
                    bxx = ip.tile([128, 9, NN], f32, tag="iD")
                    nc.vector.tensor_add(bxx, dxf, bx128)
                    bxi = ip.tile([128, 9, NN], i32, tag="iC")
                    nc.vector.tensor_copy(bxi, bxx)
                    bxf = ip.tile([128, 9, NN], f32, tag="iD")
                    nc.vector.tensor_copy(bxf, bxi)
                    idxf = ip.tile([128, 9, NN], f32, tag="iA")
                    nc.vector.tensor_scalar(idxf, ayf, 128.0, -1032.0,
                                            ALU.mult, ALU.add)
                    nc.vector.tensor_add(idxf, idxf, bxf)
                    idx_t = ip.tile([128, 9, NN], i16, tag="iE")
                    nc.vector.tensor_copy(idx_t, idxf)
                    nc.vector.tensor_scalar(idxf, idxf, 128.0, None, ALU.add)
                    idx_b = ip.tile([128, 9, NN], i16, tag="iF")
                    nc.vector.tensor_copy(idx_b, idxf)

                    # bounce: didx[u*720 + k*80 + tb*40 + n]
                    dv = didx_d[0, :].rearrange("(u k b n) -> u k b n",
                                                u=128, k=9, b=2, n=NN)
                    nc.sync.dma_start(out=dv[:, :, 0, :], in_=idx_t)
                    nc.sync.dma_start(out=dv[:, :, 1, :], in_=idx_b)
                    # req[16g+q, t, k, tb, n] = didx[(16t+q)*720 + k*80+tb*40+n]
                    rin = didx_d[0, :].rearrange("(t q c) -> q t c", t=NT, q=16)
                    for g in range(8):
                        nc.sync.dma_start(
                            out=req[16 * g:16 * g + 16, :].rearrange(
                                "q (t c) -> q t c", t=NT),
                            in_=rin)

            # ======== Stage C + D (interleaved per t) ========
            out2 = pp.tile([128, 2, NPOS], bf16)
            with tc.tile_pool(name="stagec", bufs=1) as cp, \
                 tc.tile_pool(name="gath", bufs=2) as gp, \
                 tc.tile_pool(name="gath3", bufs=3) as gp3, \
                 tc.tile_pool(name="sK", bufs=2) as skp, \
                 tc.tile_pool(name="sK1", bufs=1) as skp1, \
                 tc.tile_pool(name="psC", bufs=1, space="PSUM") as psC, \
                 tc.tile_pool(name="psW", bufs=1, space="PSUM") as psW, \
                 tc.tile_pool(name="psD", bufs=1, space="PSUM") as psD:

                tmflat = tm.rearrange("p r a b -> p (r a b)")
                segs = [(0, 512), (512, 128)]
                for t in range(NT):
                    s0s = []
                    for k in range(9):
                        r = k * NT + t
                        gt = gp3.tile([128, 4, NJ], bf16, tag="gt")
                        gb = gp3.tile([128, 4, NJ], bf16, tag="gb")
                        for tb, gdst in ((0, gt), (1, gb)):
                            nc.gpsimd.dma_gather(
                                out_ap=gdst[:, :, :],
                                in_ap=tmflat,
                                idxs_ap=req[:, t * 720 + k * 80 + tb * 40:
                                            t * 720 + k * 80 + (tb + 1) * 40],
                                num_idxs=NJ, num_idxs_reg=NJ,
                                elem_size=512, transpose=True,
                                sbuf_tokens_per_rank=128,
                                sbuf_free_dim_per_rank=1024,
                                queue_num=(2 * (t * 9 + k) + tb) % 4)
                        wsb = gp.tile([128, 4, NJ], bf16, tag="wsb",
                                      name=f"wsb_{t}_{k}")
                        for half in range(2):
                            pw = psW.tile([128, 2 * NJ], f32, tag="wps",
                                          name=f"wps_{t}_{k}_{half}")
                            rhs = maps4[:, 2 * half:2 * half + 2, :].rearrange(
                                "p a c -> p (a c)")
                            for (s0c, sn) in ((0, 512), (512, 512), (1024, 256)):
                                nc.tensor.matmul(
                                    pw[:, s0c:s0c + sn],
                                    selM[:, r * 128:(r + 1) * 128],
                                    rhs[:, s0c:s0c + sn],
                                    start=True, stop=True)
                            nc.scalar.copy(
                                wsb[:, 2 * half:2 * half + 2, :].rearrange(
                                    "p a c -> p (a c)"), pw)

                        s0 = (skp if k < 2 else skp1).tile(
                            [128, 2, NJ], bf16, tag=f"s0_{k}",
                            name=f"s0_{t}_{k}")
                        s1 = gp.tile([128, 2, NJ], bf16, tag="s1")

                        def bc(w):
                            return w.unsqueeze(1).broadcast_to([128, 2, NJ])

                        nc.vector.tensor_mul(s0, gt[:, 0:2, :],
                                             bc(wsb[:, 0, :]))
                        nc.vector.tensor_mul(s1, gt[:, 2:4, :],
                                             bc(wsb[:, 1, :]))
                        nc.vector.tensor_add(s0, s0, s1)
                        nc.vector.tensor_mul(s1, gb[:, 0:2, :],
                                             bc(wsb[:, 2, :]))
                        nc.vector.tensor_add(s0, s0, s1)
                        nc.vector.tensor_mul(s1, gb[:, 2:4, :],
                                             bc(wsb[:, 3, :]))
                        nc.vector.tensor_add(s0, s0, s1)
                        s0s.append(s0)
                    for mt in range(2):
                        acc = psC.tile([128, NJ], f32, tag="dacc",
                                       name=f"dacc_{t}_{mt}")
                        for (c0s, cn) in segs:
                            for k in range(9):
                                for ct in range(2):
                                    lhsT = w2s[:, (k * 2 + ct) * CB + mt * 128:
                                               (k * 2 + ct) * CB + (mt + 1) * 128]
                                    nc.tensor.matmul(
                                        acc[:, c0s:c0s + cn], lhsT,
                                        s0s[k][:, ct, c0s:c0s + cn],
                                        start=(k == 0 and ct == 0),
                                        stop=(k == 8 and ct == 1))
                        nc.scalar.activation(out2[:, mt, t * NJ:(t + 1) * NJ],
                                             acc, AF.Relu,
                                             bias=b2[:, mt:mt + 1])
                    # ---- Stage D for chunk t ----
                    n0 = t * NJ
                    for mt in range(8):
                        xr = iop.tile([128, NJ], bf16, tag="xres")
                        nc.sync.dma_start(out=xr,
                                          in_=xres_d[mt, :, n0:n0 + NJ])
                        ps = psD.tile([128, NJ], f32, tag="c3ps",
                                      name=f"c3ps_{t}_{mt}")
                        for (c0s, cn) in segs:
                            for ct in range(2):
                                nc.tensor.matmul(
                                    ps[:, c0s:c0s + cn],
                                    w3s[:, ct * COUT + mt * 128:
                                        ct * COUT + (mt + 1) * 128],
                                    out2[:, ct, n0 + c0s:n0 + c0s + cn],
                                    start=(ct == 0), stop=False)
                            nc.tensor.matmul(ps[:, c0s:c0s + cn], ident,
                                             xr[:, c0s:c0s + cn],
                                             start=False, stop=True)
                        o = iop.tile([128, NJ], bf16, tag="obuf")
                        nc.scalar.activation(o, ps, AF.Relu,
                                             bias=b3[:, mt:mt + 1])
                        nc.sync.dma_start(out=out_d[mt, :, n0:n0 + NJ], in_=o)

    nc.finalize()
    return nc


_NC_CACHE = None


def _get_nc():
    global _NC_CACHE
    if _NC_CACHE is None:
        _NC_CACHE = _build_program()
    return _NC_CACHE


def _perm():
    """pi-order: column t*640 + 16n + q  ->  p = 128n + 16t + q."""
    t = np.arange(NPOS) // NJ
    j = np.arange(NPOS) % NJ
    n, q = j // 16, j % 16
    return (128 * n + 16 * t + q).astype(np.int64)


def _prep_inputs(x, w1, s1, b1, w_off, b_off, w2, s2, b2, w3, s3, b3):
    bf16 = ml_dtypes.bfloat16
    f32 = np.float32
    x = np.asarray(x, f32)
    w1f = np.asarray(w1, f32) * np.asarray(s1, f32)[:, None]
    # w1T[ci, kt, co]: contraction cin = kt*128 + ci
    w1T = np.ascontiguousarray(
        w1f.T.reshape(8, 128, CB).transpose(1, 0, 2).reshape(128, 8 * CB)
    ).astype(bf16)
    w1b = np.asarray(b1, f32)[None, :].astype(bf16)
    w_off = np.asarray(w_off, f32)
    woff = np.zeros((128, 9, 2, KOFF), f32)
    for tap in range(9):
        ti, tj = divmod(tap, 3)
        wt = w_off[:, :, ti, tj]          # [27, 256]
        for ct in range(2):
            woff[:, tap, ct, :] = wt[:, ct * 128:(ct + 1) * 128].T
    woff = woff.reshape(128, 9 * 2 * KOFF).astype(bf16)
    boff = np.asarray(b_off, f32)[:, None]
    w2f = np.asarray(w2, f32) * np.asarray(s2, f32)[:, None, None, None]
    w2pk = np.zeros((128, 9, 2, CB), f32)
    for k in range(9):
        ki, kj = divmod(k, 3)
        wk = w2f[:, :, ki, kj]            # [256 out, 256 in]
        for ct in range(2):
            w2pk[:, k, ct, :] = wk[:, ct * 128:(ct + 1) * 128].T
    w2pk = w2pk.reshape(128, 9 * 2 * CB).astype(bf16)
    b2t = np.ascontiguousarray(np.asarray(b2, f32).reshape(2, 128).T)
    w3f = np.asarray(w3, f32) * np.asarray(s3, f32)[:, None]
    w3pk = np.zeros((128, 2, COUT), f32)
    for ct in range(2):
        w3pk[:, ct, :] = w3f[:, ct * 128:(ct + 1) * 128].T
    w3pk = w3pk.reshape(128, 2 * COUT).astype(bf16)
    b3t = np.ascontiguousarray(np.asarray(b3, f32).reshape(8, 128).T)

    perm = _perm()                        # col -> p
    p_lin = np.arange(NPOS)
    y_loc = np.where(p_lin < NVALID, PAD + p_lin // W, 20).astype(f32)
    x_pad = np.where(p_lin < NVALID, PAD + p_lin % W, 50).astype(f32)

    # [128, 9*40] layout: partition u, col k*40+n -> p = 128n + u
    y_w = y_loc.reshape(NN, 128).T          # [u, n]
    x_w = x_pad.reshape(NN, 128).T
    by128 = np.zeros((128, 9, NN), f32)
    bx128 = np.zeros((128, 9, NN), f32)
    for k in range(9):
        ki, kj = divmod(k, 3)
        by128[:, k, :] = y_w + np.float32(ki - 1 + 7.5)
        bx128[:, k, :] = x_w + np.float32(kj - 1 + 7.5)
    by128 = by128.reshape(128, 9 * NN)
    bx128 = bx128.reshape(128, 9 * NN)

    # [72, 640] layout: row k*8+t, col j -> p = perm[t*640+j]
    byM = np.zeros((NKT, NJ), f32)
    bxM = np.zeros((NKT, NJ), f32)
    for k in range(9):
        ki, kj = divmod(k, 3)
        for t in range(NT):
            pp_ = perm[t * NJ:(t + 1) * NJ]
            byM[k * NT + t] = y_loc[pp_] + (ki - 1) + 7.5
            bxM[k * NT + t] = x_pad[pp_] + (kj - 1) + 7.5

    selm = np.zeros((NKT, NKT * 128), bf16)
    for r in range(NKT):
        selm[r, r * 128:(r + 1) * 128] = 1.0

    shared = dict(w1T=w1T, w1b=w1b, woff=woff, boff=boff, w2=w2pk, b2=b2t,
                  w3=w3pk, b3=b3t, by128=by128, bx128=bx128,
                  byM=byM, bxM=bxM, sel=np.asarray(selm))

    in_maps = []
    for core in range(8):
        b, half = core // 2, core % 2
        lo = half * 50
        xs = np.zeros((CIN, RSTRIP, W), f32)
        vlo = max(0, lo - PAD)
        vhi = min(H - 1, lo + 49 + PAD)
        loc0 = vlo - (lo - PAD)
        nrows = vhi - vlo + 1
        xs[:, loc0:loc0 + nrows, :] = x[b, :, vlo:vhi + 1, :]
        # xs_d [128 ci, 8 kt, 5800]
        xs_t = np.ascontiguousarray(
            xs.reshape(8, 128, RSTRIP * W).transpose(1, 0, 2)
        ).reshape(128, 8 * RSTRIP * W).astype(bf16)
        indv = np.zeros((RSTRIP, W), f32)
        indv[loc0:loc0 + nrows, :] = 1.0
        vbm = np.zeros((NKT, 4), f32)
        vbm[:, 0] = loc0 + 8
        vbm[:, 1] = loc0 + nrows - 1 + 8
        vbm[:, 2] = loc0 + 8 - 1
        vbm[:, 3] = loc0 + nrows - 1 + 8 - 1
        # residual, pi-ordered bf16: xres[mt, ci, col] = x[b, mt*128+ci, p(col)]
        xflat = x[b].reshape(CIN, H * W)
        pv = perm.copy()
        src = lo * W + pv                     # absolute flat position
        valid = pv < NVALID
        xr = np.zeros((CIN, NPOS), f32)
        xr[:, valid] = xflat[:, src[valid]]
        xres = np.ascontiguousarray(
            xr.reshape(8, 128, NPOS)).astype(bf16)
        in_maps.append(dict(shared, xs=xs_t,
                            ind=indv.reshape(1, -1).astype(bf16),
                            vb=vbm, xres=xres))
    return in_maps


def kernel(**inputs):
    from concourse.bass_utils import run_bass_kernel_spmd
    nc = _get_nc()
    in_maps = _prep_inputs(**inputs)
    res = run_bass_kernel_spmd(nc, in_maps, core_ids=list(range(8)))
    perm = _perm()
    inv = np.empty(NPOS, np.int64)
    inv[perm] = np.arange(NPOS)
    out = np.zeros((B, COUT, H, W), np.float32)
    for core in range(8):
        b, half = core // 2, core % 2
        lo = half * 50
        o = res.results[core]["out"].astype(np.float32)  # [8, 128, NPOS]
        o = o.reshape(COUT, NPOS)[:, inv[:NVALID]].reshape(COUT, 50, W)
        out[b, :, lo:lo + 50, :] = o
    return out
